# revision 27
# baseline (speedup 1.0000x reference)
# Multi-head attention (B=2, S=2048, D=1024, H=16) on 8 TRN2 NeuronCores.
#
# Sharding: core c -> batch b = c//4, head group g = c%4 (4 heads = 256
# features). Each core computes its heads' attention for its batch plus the
# row-parallel slice of the output projection; the host sums the 4 partials
# per batch (the all-reduce) and adds bo.
#
# Device math per core (layouts transposed so softmax needs no cross-
# partition reduce; all matmul operands fp16, accumulation fp32 in PSUM):
#   qhT[f, s] = wq_g @ q_b^T ; khT, vhT likewise     (fp16 matmuls)
#   vh[s, f]  = PE-transpose(vhT), slot layout per head: [1 | 0pad63 | v 64]
#   logitsT[k, q] = khT_h-slices^T @ qhT_h           (K=64, pairs row-packed
#                                                     into one 2-bank psum)
#   expT = exp(logitsT / 8) * (1 - mask)^T           (one wide ACT op, DVE
#                                                     mask mult, fp16)
#   av[:, q] = vh_slot^T @ expT                      (row 0 = denominator,
#                                                     rows 64:128 = head out)
#   attnN = av * broadcast(1/denom)                  (reciprocal + K=1 ones
#                                                     outer-product matmul)
#   partial[q, D] = attnN^T @ wo[:, g-cols]^T
import os
import numpy as np

B, S, DM, H, DEPTH = 2, 2048, 1024, 16, 64
NCORES = 8
GROUPS = 4            # head-groups per batch == cores per batch
HG = H // GROUPS      # heads per core
FS = HG * DEPTH       # features per core
QC = 512              # q-block (matmul free dim)
NQC = S // QC
NKC = S // 128        # k chunks
PAIRS = HG // 2
CCH = DM // 128       # contraction chunks for the projections

_CACHE = {}


def _build():
    import concourse.tile as tile
    from concourse import bacc, mybir

    dt = mybir.dt
    f32, f16 = dt.float32, dt.float16
    Act = mybir.ActivationFunctionType

    nc = bacc.Bacc("TRN2", target_bir_lowering=False, debug=False,
                   num_devices=NCORES)

    xq = nc.dram_tensor("xq", [DM, S], f16, kind="ExternalInput").ap()
    xk = nc.dram_tensor("xk", [DM, S], f16, kind="ExternalInput").ap()
    xv = nc.dram_tensor("xv", [DM, S], f16, kind="ExternalInput").ap()
    wqd = nc.dram_tensor("wq", [DM, FS], f16, kind="ExternalInput").ap()
    wkd = nc.dram_tensor("wk", [DM, FS], f16, kind="ExternalInput").ap()
    wvd = nc.dram_tensor("wv", [DM, FS], f16, kind="ExternalInput").ap()
    wod = nc.dram_tensor("wo", [HG, DEPTH, DM], f16, kind="ExternalInput").ap()
    m01 = nc.dram_tensor("m01", [S, S], f16, kind="ExternalInput").ap()
    bqd = nc.dram_tensor("bq", [128, 2], f32, kind="ExternalInput").ap()
    bkd = nc.dram_tensor("bk", [128, 2], f32, kind="ExternalInput").ap()
    out = nc.dram_tensor("part", [S, DM], f32, kind="ExternalOutput").ap()

    with tile.TileContext(nc) as tc:
        with (
            tc.tile_pool(name="xp", bufs=4) as xp,
            tc.tile_pool(name="wp", bufs=2) as wp,
            tc.tile_pool(name="wop", bufs=4) as wop,
            tc.tile_pool(name="qk", bufs=4) as qkp,
            tc.tile_pool(name="xv", bufs=8) as xvp,
            tc.tile_pool(name="vh", bufs=16) as vp,
            tc.tile_pool(name="mk", bufs=32) as mkp,
            tc.tile_pool(name="ex", bufs=6) as exp_p,
            tc.tile_pool(name="exm", bufs=10) as exm_p,
            tc.tile_pool(name="au", bufs=4) as aup,
            tc.tile_pool(name="an", bufs=8) as anp,
            tc.tile_pool(name="rr", bufs=4) as rrp,
            tc.tile_pool(name="os", bufs=4) as osp,
            tc.tile_pool(name="cst", bufs=4) as cst,
            tc.tile_pool(name="ps", bufs=4, space="PSUM") as psp,
        ):
            def big():
                return psp.tile([128, 2, QC], f32, tag="big", name="big")

            # weights, in use-order (wq first so the PE starts ASAP)
            wq_t = wp.tile([128, CCH, FS], f16, tag="w", name="w")
            nc.sync.dma_start(wq_t[:], wqd.rearrange("(c p) f -> p c f", p=128))
            bq_t = cst.tile([128, 2], f32, tag="bias", name="bias")
            nc.sync.dma_start(bq_t[:], bqd[:])
            wk_t = wp.tile([128, CCH, FS], f16, tag="w", name="w")
            nc.sync.dma_start(wk_t[:], wkd.rearrange("(c p) f -> p c f", p=128))
            bk_t = cst.tile([128, 2], f32, tag="bias", name="bias")
            nc.sync.dma_start(bk_t[:], bkd[:])
            wv_t = wp.tile([128, CCH, FS], f16, tag="w", name="w")
            nc.sync.dma_start(wv_t[:], wvd.rearrange("(c p) f -> p c f", p=128))
            ones_row = cst.tile([1, 128], f16, tag="ones", name="ones")
            nc.vector.memset(ones_row[:], 1.0)
            wo_t = []
            for h in range(HG):
                t = wop.tile([DEPTH, DM], f16, tag="wo", name="wo")
                nc.sync.dma_start(t[:], wod[h])
                wo_t.append(t)

            # ---- projections (q/k produce transposed [FS, S] fp16) ----
            qhT = [qkp.tile([128, S], f16, tag="qk", name="qk") for _ in range(2)]
            khT = [qkp.tile([128, S], f16, tag="qk", name="qk") for _ in range(2)]

            def proj(xd, w_t, dst, bias):
                pst = [big() for _ in range(4)]
                for c in range(CCH):
                    xt = xp.tile([128, S], f16, tag="x", name="x")
                    nc.sync.dma_start(xt[:], xd[128 * c:128 * (c + 1), :])
                    for m in range(2):
                        for qs in range(4):
                            i = m * 4 + qs
                            nc.tensor.matmul(
                                pst[i // 2][:, i % 2, :],
                                lhsT=w_t[:, c, 128 * m:128 * (m + 1)],
                                rhs=xt[:, QC * qs:QC * (qs + 1)],
                                start=(c == 0), stop=(c == CCH - 1),
                            )
                # copybacks on ScalarE: it is idle during the projection
                # phase while VectorE is the attention-phase co-bottleneck
                for m in range(2):
                    for qs in range(4):
                        i = m * 4 + qs
                        nc.scalar.add(
                            dst[m][:, QC * qs:QC * (qs + 1)],
                            pst[i // 2][:, i % 2, :], bias[:, m:m + 1])

            proj(xq, wq_t, qhT, bq_t)
            proj(xk, wk_t, khT, bk_t)

            # v projection straight into vh [k, feat] layout: kr-outer with
            # xv resident, one transient psum bank at a time (so it overlaps
            # with the start of attention).  vh slot layout (128 wide): col 0
            # = ones (denominator), cols 1:64 zero pad, cols 64:128 = v data
            # -> av row 0 = denominator, rows 64:128 = head output.
            xv_t = []
            for c in range(CCH):
                t = xvp.tile([128, S], f16, tag="xv", name="xv")
                nc.sync.dma_start(t[:], xv[128 * c:128 * (c + 1), :])
                xv_t.append(t)
            vh = [vp.tile([128, HG, 128], f16, tag="vh", name="vh")
                  for _ in range(NKC)]
            for kr in range(NKC):
                pv = psp.tile([128, 256], f32, tag="big", name="big")
                for c in range(CCH):
                    nc.tensor.matmul(
                        pv[:], lhsT=xv_t[c][:, 128 * kr:128 * (kr + 1)],
                        rhs=wv_t[:, c, :],
                        start=(c == 0), stop=(c == CCH - 1))
                nc.vector.memset(vh[kr][:, :, 0:1], 1.0)
                nc.vector.memset(vh[kr][:, :, 1:64], 0.0)
                nc.scalar.copy(
                    vh[kr][:, :, 64:128],
                    pv.rearrange("p (h d) -> p h d", d=DEPTH))

            # ---- attention + output projection, per q-block ----
            def emit_wo(qcb, attnN):
                for qm in range(QC // 128):
                    row = slice(128 * (4 * qcb + qm), 128 * (4 * qcb + qm + 1))
                    po = big()
                    for dn in range(2):
                        dsl = slice(512 * dn, 512 * (dn + 1))
                        for h in range(HG):
                            nc.tensor.matmul(
                                po[:, dn, :],
                                lhsT=attnN[h][:, 128 * qm:128 * (qm + 1)],
                                rhs=wo_t[h][:, dsl],
                                start=(h == 0), stop=(h == HG - 1))
                    ot = osp.tile([128, 2, 512], f32, tag="os", name="os")
                    nc.scalar.copy(ot[:, 0, :], po[:, 0, :])
                    nc.vector.tensor_copy(ot[:, 1, :], po[:, 1, :])
                    nc.sync.dma_start(
                        out[row, :].rearrange("p (o q) -> p o q", o=2), ot[:])

            prev_wo = None
            for qcb in range(NQC):
                qsl = slice(QC * qcb, QC * (qcb + 1))
                mk = []
                for kc in range(NKC):
                    t = mkp.tile([128, QC], f16, tag="mk", name="mk")
                    nc.sync.dma_start(
                        t[:], m01[128 * kc:128 * (kc + 1), qsl])
                    mk.append(t)

                av2 = [big() for _ in range(PAIRS)]   # halves = heads A/B

                def emit_av(pair, dk, exm2, av2=av2):
                    for half in range(2):
                        nc.tensor.matmul(
                            av2[pair][:, half, :],
                            lhsT=vh[dk][:, 2 * pair + half, :],
                            rhs=exm2[:, half, :],
                            start=(dk == 0), stop=(dk == NKC - 1),
                            skip_group_check=True)

                def normalize(pair, av2=av2):
                    res = {}
                    for half in range(2):
                        h = 2 * pair + half
                        au = aup.tile([64, QC], f32, tag="au", name="au")
                        nc.vector.tensor_copy(au[:], av2[pair][64:128, half, :])
                        rr = rrp.tile([1, QC], f32, tag="rr", name="rr")
                        nc.vector.reciprocal_approx_fast(
                            rr[:], av2[pair][0:1, half, :])
                        rr16 = rrp.tile([1, QC], f16, tag="rr16", name="rr16")
                        nc.vector.tensor_copy(rr16[:], rr[:])
                        rbc = psp.tile([128, QC], f32, tag="big", name="big")
                        nc.tensor.matmul(
                            rbc[:], lhsT=ones_row[:], rhs=rr16[:],
                            start=True, stop=True)
                        an = anp.tile([64, QC], f16, tag="an", name="an")
                        nc.vector.tensor_mul(an[:], au[:], rbc[0:64, :])
                        res[h] = an
                    return res

                # Both pairs' pipelines interleaved; AV trails logits by 4
                # k-chunks so the PE never stalls on the exp/mask pipeline.
                # The previous q-block's output projection is emitted in the
                # middle of this block's pipeline so it fills PE gaps instead
                # of serializing at the block boundary.
                # Pairs run sequentially: pair 1's logits/exp pipeline
                # overlaps pair 0's normalize chain (DVE), and the single
                # active pair gets deeper logits-psum buffering.
                attnN = {}
                for pair in range(PAIRS):
                    pend = []
                    for kc in range(NKC):
                        ksl = slice(128 * kc, 128 * (kc + 1))
                        lg2 = big()
                        for half in range(2):
                            psl = slice(64 * half, 64 * (half + 1))
                            nc.tensor.matmul(
                                lg2[:, half, :],
                                lhsT=khT[pair][psl, ksl],
                                rhs=qhT[pair][psl, qsl],
                                start=True, stop=True)
                        ex2 = exp_p.tile([128, 2, QC], f16, tag="ex", name="ex")
                        nc.scalar.activation(
                            ex2[:], lg2[:], Act.Exp, scale=0.125)
                        exm2 = exm_p.tile([128, 2, QC], f16, tag="exm",
                                          name="exm")
                        nc.vector.tensor_mul(
                            exm2[:], ex2[:],
                            mk[kc][:].rearrange("p (o q) -> p o q", o=1)
                            .to_broadcast((128, 2, QC)))
                        pend.append((pair, kc, exm2))
                        if len(pend) > 4:
                            p_, dk, dexm = pend.pop(0)
                            emit_av(p_, dk, dexm)
                        if pair == 0 and kc == 6 and prev_wo is not None:
                            emit_wo(*prev_wo)
                            prev_wo = None
                    for p_, dk, dexm in pend:
                        emit_av(p_, dk, dexm)
                    attnN.update(normalize(pair))
                prev_wo = (qcb, attnN)
            emit_wo(*prev_wo)

    nc.compile()
    return nc


def _get_program():
    if "nc" not in _CACHE:
        _CACHE["nc"] = _build()
    return _CACHE["nc"]


def _in_maps(q, k, v, mask, wq, bq, wk, bk, wv, bv, wo, bo):
    q = np.asarray(q, np.float32)
    k = np.asarray(k, np.float32)
    v = np.asarray(v, np.float32)
    mask = np.asarray(mask, np.float32)
    wq = np.asarray(wq, np.float32)
    wk = np.asarray(wk, np.float32)
    wv = np.asarray(wv, np.float32)
    wo = np.asarray(wo, np.float32)
    bq = np.asarray(bq, np.float32)
    bk = np.asarray(bk, np.float32)
    bv = np.asarray(bv, np.float32)
    assert np.all(bv == 0.0), "nonzero bv not supported by this kernel"

    maps = []
    xqT = [np.ascontiguousarray(q[b].T).astype(np.float16) for b in range(B)]
    xkT = [np.ascontiguousarray(k[b].T).astype(np.float16) for b in range(B)]
    xvT = [np.ascontiguousarray(v[b].T).astype(np.float16) for b in range(B)]
    m01 = [np.ascontiguousarray((1.0 - mask[b, 0]).T).astype(np.float16)
           for b in range(B)]
    for c in range(NCORES):
        b, g = divmod(c, GROUPS)
        cols = slice(FS * g, FS * (g + 1))
        maps.append({
            "xq": xqT[b], "xk": xkT[b], "xv": xvT[b],
            "wq": np.ascontiguousarray(wq[cols].T).astype(np.float16),
            "wk": np.ascontiguousarray(wk[cols].T).astype(np.float16),
            "wv": np.ascontiguousarray(wv[cols].T).astype(np.float16),
            "wo": np.ascontiguousarray(
                wo[:, cols].T.reshape(HG, DEPTH, DM)).astype(np.float16),
            "m01": m01[b],
            "bq": np.ascontiguousarray(bq[cols].reshape(2, 128).T),
            "bk": np.ascontiguousarray(bk[cols].reshape(2, 128).T),
        })
    return maps


def _run(maps, trace=False):
    from concourse.bass_utils import run_bass_kernel_spmd
    nc = _get_program()
    kwargs = {}
    if trace:
        kwargs = dict(trace=True, tmpdir=os.environ.get("KERNEL_TRACE_DIR"))
    return run_bass_kernel_spmd(nc, maps, list(range(NCORES)), **kwargs)


def kernel(q, k, v, mask, wq, bq, wk, bk, wv, bv, wo, bo):
    maps = _in_maps(q, k, v, mask, wq, bq, wk, bk, wv, bv, wo, bo)
    res = _run(maps)
    parts = [res.results[c]["part"] for c in range(NCORES)]
    bo = np.asarray(bo, np.float32)
    outb = [parts[GROUPS * b] + parts[GROUPS * b + 1]
            + parts[GROUPS * b + 2] + parts[GROUPS * b + 3] + bo
            for b in range(B)]
    return np.stack(outb, 0).astype(np.float32)


# revision 28
# speedup vs baseline: 1.0117x; 1.0117x over previous
# Multi-head attention (B=2, S=2048, D=1024, H=16) on 8 TRN2 NeuronCores.
#
# Sharding: core c -> batch b = c//4, head group g = c%4 (4 heads = 256
# features). Each core computes its heads' attention for its batch plus the
# row-parallel slice of the output projection; the host sums the 4 partials
# per batch (the all-reduce) and adds bo.
#
# Device math per core (layouts transposed so softmax needs no cross-
# partition reduce; all matmul operands fp16, accumulation fp32 in PSUM):
#   qhT[f, s] = wq_g @ q_b^T ; khT, vhT likewise     (fp16 matmuls)
#   vh[s, f]  = PE-transpose(vhT), slot layout per head: [1 | 0pad63 | v 64]
#   logitsT[k, q] = khT_h-slices^T @ qhT_h           (K=64, pairs row-packed
#                                                     into one 2-bank psum)
#   expT = exp(logitsT / 8) * (1 - mask)^T           (one wide ACT op, DVE
#                                                     mask mult, fp16)
#   av[:, q] = vh_slot^T @ expT                      (row 0 = denominator,
#                                                     rows 64:128 = head out)
#   attnN = av * broadcast(1/denom)                  (reciprocal + K=1 ones
#                                                     outer-product matmul)
#   partial[q, D] = attnN^T @ wo[:, g-cols]^T
import os
import numpy as np

B, S, DM, H, DEPTH = 2, 2048, 1024, 16, 64
NCORES = 8
GROUPS = 4            # head-groups per batch == cores per batch
HG = H // GROUPS      # heads per core
FS = HG * DEPTH       # features per core
QC = 512              # q-block (matmul free dim)
NQC = S // QC
NKC = S // 128        # k chunks
PAIRS = HG // 2
CCH = DM // 128       # contraction chunks for the projections

_CACHE = {}


def _build():
    import concourse.tile as tile
    from concourse import bacc, mybir

    dt = mybir.dt
    f32, f16 = dt.float32, dt.float16
    Act = mybir.ActivationFunctionType

    nc = bacc.Bacc("TRN2", target_bir_lowering=False, debug=False,
                   num_devices=NCORES)

    xq = nc.dram_tensor("xq", [DM, S], f16, kind="ExternalInput").ap()
    xk = nc.dram_tensor("xk", [DM, S], f16, kind="ExternalInput").ap()
    xv = nc.dram_tensor("xv", [DM, S], f16, kind="ExternalInput").ap()
    wqd = nc.dram_tensor("wq", [DM, FS], f16, kind="ExternalInput").ap()
    wkd = nc.dram_tensor("wk", [DM, FS], f16, kind="ExternalInput").ap()
    wvd = nc.dram_tensor("wv", [DM, FS], f16, kind="ExternalInput").ap()
    wod = nc.dram_tensor("wo", [HG, DEPTH, DM], f16, kind="ExternalInput").ap()
    m01 = nc.dram_tensor("m01", [S, S], f16, kind="ExternalInput").ap()
    bqd = nc.dram_tensor("bq", [128, 2], f32, kind="ExternalInput").ap()
    bkd = nc.dram_tensor("bk", [128, 2], f32, kind="ExternalInput").ap()
    out = nc.dram_tensor("part", [S, DM], f32, kind="ExternalOutput").ap()

    with tile.TileContext(nc) as tc:
        with (
            tc.tile_pool(name="xp", bufs=4) as xp,
            tc.tile_pool(name="wp", bufs=2) as wp,
            tc.tile_pool(name="wop", bufs=4) as wop,
            tc.tile_pool(name="qk", bufs=4) as qkp,
            tc.tile_pool(name="xv", bufs=8) as xvp,
            tc.tile_pool(name="vh", bufs=16) as vp,
            tc.tile_pool(name="mk", bufs=32) as mkp,
            tc.tile_pool(name="ex", bufs=6) as exp_p,
            tc.tile_pool(name="exm", bufs=10) as exm_p,
            tc.tile_pool(name="au", bufs=4) as aup,
            tc.tile_pool(name="an", bufs=8) as anp,
            tc.tile_pool(name="rr", bufs=4) as rrp,
            tc.tile_pool(name="os", bufs=4) as osp,
            tc.tile_pool(name="cst", bufs=4) as cst,
            tc.tile_pool(name="ps", bufs=4, space="PSUM") as psp,
        ):
            def big():
                return psp.tile([128, 2, QC], f32, tag="big", name="big")

            # weights, in use-order (wq first so the PE starts ASAP)
            wq_t = wp.tile([128, CCH, FS], f16, tag="w", name="w")
            nc.sync.dma_start(wq_t[:], wqd.rearrange("(c p) f -> p c f", p=128))
            bq_t = cst.tile([128, 2], f32, tag="bias", name="bias")
            nc.sync.dma_start(bq_t[:], bqd[:])
            wk_t = wp.tile([128, CCH, FS], f16, tag="w", name="w")
            nc.sync.dma_start(wk_t[:], wkd.rearrange("(c p) f -> p c f", p=128))
            bk_t = cst.tile([128, 2], f32, tag="bias", name="bias")
            nc.sync.dma_start(bk_t[:], bkd[:])
            wv_t = wp.tile([128, CCH, FS], f16, tag="w", name="w")
            nc.sync.dma_start(wv_t[:], wvd.rearrange("(c p) f -> p c f", p=128))
            ones_row = cst.tile([1, 128], f16, tag="ones", name="ones")
            nc.vector.memset(ones_row[:], 1.0)
            wo_t = []
            for h in range(HG):
                t = wop.tile([DEPTH, DM], f16, tag="wo", name="wo")
                nc.sync.dma_start(t[:], wod[h])
                wo_t.append(t)

            # ---- projections (q/k produce transposed [FS, S] fp16) ----
            qhT = [qkp.tile([128, S], f16, tag="qk", name="qk") for _ in range(2)]
            khT = [qkp.tile([128, S], f16, tag="qk", name="qk") for _ in range(2)]

            def proj(xd, w_t, dst, bias):
                pst = [big() for _ in range(4)]
                for c in range(CCH):
                    xt = xp.tile([128, S], f16, tag="x", name="x")
                    nc.sync.dma_start(xt[:], xd[128 * c:128 * (c + 1), :])
                    for m in range(2):
                        for qs in range(4):
                            i = m * 4 + qs
                            nc.tensor.matmul(
                                pst[i // 2][:, i % 2, :],
                                lhsT=w_t[:, c, 128 * m:128 * (m + 1)],
                                rhs=xt[:, QC * qs:QC * (qs + 1)],
                                start=(c == 0), stop=(c == CCH - 1),
                            )
                # copybacks on ScalarE: it is idle during the projection
                # phase while VectorE is the attention-phase co-bottleneck
                for m in range(2):
                    for qs in range(4):
                        i = m * 4 + qs
                        nc.scalar.add(
                            dst[m][:, QC * qs:QC * (qs + 1)],
                            pst[i // 2][:, i % 2, :], bias[:, m:m + 1])

            proj(xq, wq_t, qhT, bq_t)
            proj(xk, wk_t, khT, bk_t)

            # v projection straight into vh [k, feat] layout: kr-outer with
            # xv resident, one transient psum bank at a time (so it overlaps
            # with the start of attention).  vh slot layout (128 wide): col 0
            # = ones (denominator), cols 1:64 zero pad, cols 64:128 = v data
            # -> av row 0 = denominator, rows 64:128 = head output.
            xv_t = []
            for c in range(CCH):
                t = xvp.tile([128, S], f16, tag="xv", name="xv")
                nc.sync.dma_start(t[:], xv[128 * c:128 * (c + 1), :])
                xv_t.append(t)
            vh = [vp.tile([128, HG, 128], f16, tag="vh", name="vh")
                  for _ in range(NKC)]
            for kr in range(NKC):
                pv = psp.tile([128, 256], f32, tag="big", name="big")
                for c in range(CCH):
                    nc.tensor.matmul(
                        pv[:], lhsT=xv_t[c][:, 128 * kr:128 * (kr + 1)],
                        rhs=wv_t[:, c, :],
                        start=(c == 0), stop=(c == CCH - 1))
                nc.vector.memset(vh[kr][:, :, 0:1], 1.0)
                nc.vector.memset(vh[kr][:, :, 1:64], 0.0)
                nc.scalar.copy(
                    vh[kr][:, :, 64:128],
                    pv.rearrange("p (h d) -> p h d", d=DEPTH))

            # ---- attention + output projection, per q-block ----
            def emit_wo(qcb, attnN):
                for qm in range(QC // 128):
                    row = slice(128 * (4 * qcb + qm), 128 * (4 * qcb + qm + 1))
                    po = big()
                    for dn in range(2):
                        dsl = slice(512 * dn, 512 * (dn + 1))
                        for h in range(HG):
                            nc.tensor.matmul(
                                po[:, dn, :],
                                lhsT=attnN[h][:, 128 * qm:128 * (qm + 1)],
                                rhs=wo_t[h][:, dsl],
                                start=(h == 0), stop=(h == HG - 1))
                    ot = osp.tile([128, 2, 512], f32, tag="os", name="os")
                    nc.scalar.copy(ot[:, 0, :], po[:, 0, :])
                    nc.vector.tensor_copy(ot[:, 1, :], po[:, 1, :])
                    nc.sync.dma_start(
                        out[row, :].rearrange("p (o q) -> p o q", o=2), ot[:])

            prev_wo = None
            for qcb in range(NQC):
                qsl = slice(QC * qcb, QC * (qcb + 1))
                mk = []
                for kc in range(NKC):
                    t = mkp.tile([128, QC], f16, tag="mk", name="mk")
                    nc.sync.dma_start(
                        t[:], m01[128 * kc:128 * (kc + 1), qsl])
                    mk.append(t)

                av2 = [big() for _ in range(PAIRS)]   # halves = heads A/B

                def emit_av(pair, dk, exm2, av2=av2):
                    for half in range(2):
                        nc.tensor.matmul(
                            av2[pair][:, half, :],
                            lhsT=vh[dk][:, 2 * pair + half, :],
                            rhs=exm2[:, half, :],
                            start=(dk == 0), stop=(dk == NKC - 1),
                            skip_group_check=True)

                def normalize(pair, av2=av2):
                    res = {}
                    for half in range(2):
                        h = 2 * pair + half
                        au = aup.tile([64, QC], f32, tag="au", name="au")
                        nc.vector.tensor_copy(au[:], av2[pair][64:128, half, :])
                        rr = rrp.tile([1, QC], f32, tag="rr", name="rr")
                        nc.vector.reciprocal_approx_fast(
                            rr[:], av2[pair][0:1, half, :])
                        rr16 = rrp.tile([1, QC], f16, tag="rr16", name="rr16")
                        nc.vector.tensor_copy(rr16[:], rr[:])
                        rbc = psp.tile([128, QC], f32, tag="big", name="big")
                        nc.tensor.matmul(
                            rbc[:], lhsT=ones_row[:], rhs=rr16[:],
                            start=True, stop=True)
                        an = anp.tile([64, QC], f16, tag="an", name="an")
                        nc.vector.tensor_mul(an[:], au[:], rbc[0:64, :])
                        res[h] = an
                    return res

                # Both pairs' pipelines interleaved; AV trails logits by 4
                # k-chunks so the PE never stalls on the exp/mask pipeline.
                # The previous q-block's output projection is emitted in the
                # middle of this block's pipeline so it fills PE gaps instead
                # of serializing at the block boundary.
                pend = {p: [] for p in range(PAIRS)}
                for kc in range(NKC):
                    ksl = slice(128 * kc, 128 * (kc + 1))
                    for pair in range(PAIRS):
                        lg2 = big()
                        for half in range(2):
                            psl = slice(64 * half, 64 * (half + 1))
                            nc.tensor.matmul(
                                lg2[:, half, :],
                                lhsT=khT[pair][psl, ksl],
                                rhs=qhT[pair][psl, qsl],
                                start=True, stop=True)
                        ex2 = exp_p.tile([128, 2, QC], f16, tag="ex", name="ex")
                        nc.scalar.activation(
                            ex2[:], lg2[:], Act.Exp, scale=0.125)
                        exm2 = exm_p.tile([128, 2, QC], f16, tag="exm",
                                          name="exm")
                        nc.vector.tensor_mul(
                            exm2[:], ex2[:],
                            mk[kc][:].rearrange("p (o q) -> p o q", o=1)
                            .to_broadcast((128, 2, QC)))
                        pend[pair].append((pair, kc, exm2))
                        if len(pend[pair]) > 4:
                            p_, dk, dexm = pend[pair].pop(0)
                            emit_av(p_, dk, dexm)
                    if kc == 6 and prev_wo is not None:
                        emit_wo(*prev_wo)
                        prev_wo = None
                attnN = {}
                for p in range(PAIRS):
                    for p_, dk, dexm in pend[p]:
                        emit_av(p_, dk, dexm)
                    attnN.update(normalize(p))
                prev_wo = (qcb, attnN)
            emit_wo(*prev_wo)

    nc.compile()
    return nc


def _get_program():
    if "nc" not in _CACHE:
        _CACHE["nc"] = _build()
    return _CACHE["nc"]


def _in_maps(q, k, v, mask, wq, bq, wk, bk, wv, bv, wo, bo):
    q = np.asarray(q, np.float32)
    k = np.asarray(k, np.float32)
    v = np.asarray(v, np.float32)
    mask = np.asarray(mask, np.float32)
    wq = np.asarray(wq, np.float32)
    wk = np.asarray(wk, np.float32)
    wv = np.asarray(wv, np.float32)
    wo = np.asarray(wo, np.float32)
    bq = np.asarray(bq, np.float32)
    bk = np.asarray(bk, np.float32)
    bv = np.asarray(bv, np.float32)
    assert np.all(bv == 0.0), "nonzero bv not supported by this kernel"

    maps = []
    xqT = [np.ascontiguousarray(q[b].T).astype(np.float16) for b in range(B)]
    xkT = [np.ascontiguousarray(k[b].T).astype(np.float16) for b in range(B)]
    xvT = [np.ascontiguousarray(v[b].T).astype(np.float16) for b in range(B)]
    m01 = [np.ascontiguousarray((1.0 - mask[b, 0]).T).astype(np.float16)
           for b in range(B)]
    for c in range(NCORES):
        b, g = divmod(c, GROUPS)
        cols = slice(FS * g, FS * (g + 1))
        maps.append({
            "xq": xqT[b], "xk": xkT[b], "xv": xvT[b],
            "wq": np.ascontiguousarray(wq[cols].T).astype(np.float16),
            "wk": np.ascontiguousarray(wk[cols].T).astype(np.float16),
            "wv": np.ascontiguousarray(wv[cols].T).astype(np.float16),
            "wo": np.ascontiguousarray(
                wo[:, cols].T.reshape(HG, DEPTH, DM)).astype(np.float16),
            "m01": m01[b],
            "bq": np.ascontiguousarray(bq[cols].reshape(2, 128).T),
            "bk": np.ascontiguousarray(bk[cols].reshape(2, 128).T),
        })
    return maps


def _run(maps, trace=False):
    from concourse.bass_utils import run_bass_kernel_spmd
    nc = _get_program()
    kwargs = {}
    if trace:
        kwargs = dict(trace=True, tmpdir=os.environ.get("KERNEL_TRACE_DIR"))
    return run_bass_kernel_spmd(nc, maps, list(range(NCORES)), **kwargs)


def kernel(q, k, v, mask, wq, bq, wk, bk, wv, bv, wo, bo):
    maps = _in_maps(q, k, v, mask, wq, bq, wk, bk, wv, bv, wo, bo)
    res = _run(maps)
    parts = [res.results[c]["part"] for c in range(NCORES)]
    bo = np.asarray(bo, np.float32)
    outb = [parts[GROUPS * b] + parts[GROUPS * b + 1]
            + parts[GROUPS * b + 2] + parts[GROUPS * b + 3] + bo
            for b in range(B)]
    return np.stack(outb, 0).astype(np.float32)


# revision 29
# speedup vs baseline: 1.0269x; 1.0150x over previous
# Multi-head attention (B=2, S=2048, D=1024, H=16) on 8 TRN2 NeuronCores.
#
# Sharding: core c -> batch b = c//4, head group g = c%4 (4 heads = 256
# features). Each core computes its heads' attention for its batch plus the
# row-parallel slice of the output projection; the host sums the 4 partials
# per batch (the all-reduce) and adds bo.
#
# Device math per core (layouts transposed so softmax needs no cross-
# partition reduce; all matmul operands fp16, accumulation fp32 in PSUM):
#   qhT[f, s] = wq_g @ q_b^T ; khT, vhT likewise     (fp16 matmuls)
#   vh[s, f]  = PE-transpose(vhT), slot layout per head: [1 | 0pad63 | v 64]
#   logitsT[k, q] = khT_h-slices^T @ qhT_h           (K=64, pairs row-packed
#                                                     into one 2-bank psum)
#   expT = exp(logitsT / 8) * (1 - mask)^T           (one wide ACT op, DVE
#                                                     mask mult, fp16)
#   av[:, q] = vh_slot^T @ expT                      (row 0 = denominator,
#                                                     rows 64:128 = head out)
#   attnN = av * broadcast(1/denom)                  (reciprocal + K=1 ones
#                                                     outer-product matmul)
#   partial[q, D] = attnN^T @ wo[:, g-cols]^T
import os
import numpy as np

B, S, DM, H, DEPTH = 2, 2048, 1024, 16, 64
NCORES = 8
GROUPS = 4            # head-groups per batch == cores per batch
HG = H // GROUPS      # heads per core
FS = HG * DEPTH       # features per core
QC = 512              # q-block (matmul free dim)
NQC = S // QC
NKC = S // 128        # k chunks
PAIRS = HG // 2
CCH = DM // 128       # contraction chunks for the projections

_CACHE = {}


def _build():
    import concourse.tile as tile
    from concourse import bacc, mybir

    dt = mybir.dt
    f32, f16 = dt.float32, dt.float16
    Act = mybir.ActivationFunctionType

    nc = bacc.Bacc("TRN2", target_bir_lowering=False, debug=False,
                   num_devices=NCORES)

    xq = nc.dram_tensor("xq", [DM, S], f16, kind="ExternalInput").ap()
    xk = nc.dram_tensor("xk", [DM, S], f16, kind="ExternalInput").ap()
    xv = nc.dram_tensor("xv", [DM, S], f16, kind="ExternalInput").ap()
    wqd = nc.dram_tensor("wq", [DM, FS], f16, kind="ExternalInput").ap()
    wkd = nc.dram_tensor("wk", [DM, FS], f16, kind="ExternalInput").ap()
    wvd = nc.dram_tensor("wv", [DM, FS], f16, kind="ExternalInput").ap()
    wod = nc.dram_tensor("wo", [HG, DEPTH, DM], f16, kind="ExternalInput").ap()
    m01 = nc.dram_tensor("m01", [S, S], f16, kind="ExternalInput").ap()
    bqd = nc.dram_tensor("bq", [128, 2], f32, kind="ExternalInput").ap()
    bkd = nc.dram_tensor("bk", [128, 2], f32, kind="ExternalInput").ap()
    out = nc.dram_tensor("part", [S, DM], f32, kind="ExternalOutput").ap()

    with tile.TileContext(nc) as tc:
        with (
            tc.tile_pool(name="xp", bufs=4) as xp,
            tc.tile_pool(name="wp", bufs=2) as wp,
            tc.tile_pool(name="wop", bufs=4) as wop,
            tc.tile_pool(name="qk", bufs=4) as qkp,
            tc.tile_pool(name="xv", bufs=8) as xvp,
            tc.tile_pool(name="vh", bufs=16) as vp,
            tc.tile_pool(name="mk", bufs=32) as mkp,
            tc.tile_pool(name="ex", bufs=6) as exp_p,
            tc.tile_pool(name="exm", bufs=10) as exm_p,
            tc.tile_pool(name="au", bufs=4) as aup,
            tc.tile_pool(name="an", bufs=8) as anp,
            tc.tile_pool(name="rr", bufs=4) as rrp,
            tc.tile_pool(name="os", bufs=4) as osp,
            tc.tile_pool(name="cst", bufs=4) as cst,
            tc.tile_pool(name="ps", bufs=4, space="PSUM") as psp,
        ):
            def big():
                return psp.tile([128, 2, QC], f32, tag="big", name="big")

            # weights, in use-order (wq first so the PE starts ASAP)
            wq_t = wp.tile([128, CCH, FS], f16, tag="w", name="w")
            nc.sync.dma_start(wq_t[:], wqd.rearrange("(c p) f -> p c f", p=128))
            bq_t = cst.tile([128, 2], f32, tag="bias", name="bias")
            nc.sync.dma_start(bq_t[:], bqd[:])
            wk_t = wp.tile([128, CCH, FS], f16, tag="w", name="w")
            nc.sync.dma_start(wk_t[:], wkd.rearrange("(c p) f -> p c f", p=128))
            bk_t = cst.tile([128, 2], f32, tag="bias", name="bias")
            nc.sync.dma_start(bk_t[:], bkd[:])
            wv_t = wp.tile([128, CCH, FS], f16, tag="w", name="w")
            nc.sync.dma_start(wv_t[:], wvd.rearrange("(c p) f -> p c f", p=128))
            ones_row = cst.tile([1, 128], f16, tag="ones", name="ones")
            nc.vector.memset(ones_row[:], 1.0)
            wo_t = []
            for h in range(HG):
                t = wop.tile([DEPTH, DM], f16, tag="wo", name="wo")
                nc.sync.dma_start(t[:], wod[h])
                wo_t.append(t)

            # ---- projections (q/k produce transposed [FS, S] fp16) ----
            qhT = [qkp.tile([128, S], f16, tag="qk", name="qk") for _ in range(2)]
            khT = [qkp.tile([128, S], f16, tag="qk", name="qk") for _ in range(2)]

            def proj(xd, w_t, dst, bias):
                pst = [big() for _ in range(4)]
                for c in range(CCH):
                    xt = xp.tile([128, S], f16, tag="x", name="x")
                    nc.sync.dma_start(xt[:], xd[128 * c:128 * (c + 1), :])
                    for m in range(2):
                        for qs in range(4):
                            i = m * 4 + qs
                            nc.tensor.matmul(
                                pst[i // 2][:, i % 2, :],
                                lhsT=w_t[:, c, 128 * m:128 * (m + 1)],
                                rhs=xt[:, QC * qs:QC * (qs + 1)],
                                start=(c == 0), stop=(c == CCH - 1),
                            )
                # copybacks on ScalarE: it is idle during the projection
                # phase while VectorE is the attention-phase co-bottleneck
                for m in range(2):
                    for qs in range(4):
                        i = m * 4 + qs
                        nc.scalar.add(
                            dst[m][:, QC * qs:QC * (qs + 1)],
                            pst[i // 2][:, i % 2, :], bias[:, m:m + 1])

            proj(xq, wq_t, qhT, bq_t)
            proj(xk, wk_t, khT, bk_t)

            # v projection straight into vh [k, feat] layout: kr-outer with
            # xv resident, one transient psum bank at a time (so it overlaps
            # with the start of attention).  vh slot layout (128 wide): col 0
            # = ones (denominator), cols 1:64 zero pad, cols 64:128 = v data
            # -> av row 0 = denominator, rows 64:128 = head output.
            xv_t = []
            for c in range(CCH):
                t = xvp.tile([128, S], f16, tag="xv", name="xv")
                nc.sync.dma_start(t[:], xv[128 * c:128 * (c + 1), :])
                xv_t.append(t)
            vh = [vp.tile([128, HG, 128], f16, tag="vh", name="vh")
                  for _ in range(NKC)]
            for kr in range(NKC):
                pv = psp.tile([128, 256], f32, tag="big", name="big")
                for c in range(CCH):
                    nc.tensor.matmul(
                        pv[:], lhsT=xv_t[c][:, 128 * kr:128 * (kr + 1)],
                        rhs=wv_t[:, c, :],
                        start=(c == 0), stop=(c == CCH - 1))
                nc.vector.memset(vh[kr][:, :, 0:1], 1.0)
                nc.vector.memset(vh[kr][:, :, 1:64], 0.0)
                nc.scalar.copy(
                    vh[kr][:, :, 64:128],
                    pv.rearrange("p (h d) -> p h d", d=DEPTH))

            # ---- attention + output projection, per q-block ----
            def emit_wo(qcb, attnN):
                for qm in range(QC // 128):
                    row = slice(128 * (4 * qcb + qm), 128 * (4 * qcb + qm + 1))
                    po = big()
                    for dn in range(2):
                        dsl = slice(512 * dn, 512 * (dn + 1))
                        for h in range(HG):
                            nc.tensor.matmul(
                                po[:, dn, :],
                                lhsT=attnN[h][:, 128 * qm:128 * (qm + 1)],
                                rhs=wo_t[h][:, dsl],
                                start=(h == 0), stop=(h == HG - 1))
                    ot = osp.tile([128, 2, 512], f32, tag="os", name="os")
                    nc.scalar.copy(ot[:, 0, :], po[:, 0, :])
                    nc.vector.tensor_copy(ot[:, 1, :], po[:, 1, :])
                    nc.sync.dma_start(
                        out[row, :].rearrange("p (o q) -> p o q", o=2), ot[:])

            prev_wo = None
            for qcb in range(NQC):
                qsl = slice(QC * qcb, QC * (qcb + 1))
                mk = []
                for kc in range(NKC):
                    t = mkp.tile([128, QC], f16, tag="mk", name="mk")
                    nc.sync.dma_start(
                        t[:], m01[128 * kc:128 * (kc + 1), qsl])
                    mk.append(t)

                av2 = [big() for _ in range(PAIRS)]   # halves = heads A/B

                def emit_av(pair, dk, exm2, av2=av2):
                    for half in range(2):
                        nc.tensor.matmul(
                            av2[pair][:, half, :],
                            lhsT=vh[dk][:, 2 * pair + half, :],
                            rhs=exm2[:, half, :],
                            start=(dk == 0), stop=(dk == NKC - 1),
                            skip_group_check=True)

                def normalize(pair, av2=av2):
                    res = {}
                    for half in range(2):
                        h = 2 * pair + half
                        # copies on ScalarE: it idles at the q-block boundary
                        # while VectorE's normalize chain is the bottleneck
                        au = aup.tile([64, QC], f32, tag="au", name="au")
                        nc.scalar.copy(au[:], av2[pair][64:128, half, :])
                        rr = rrp.tile([1, QC], f32, tag="rr", name="rr")
                        nc.vector.reciprocal_approx_fast(
                            rr[:], av2[pair][0:1, half, :])
                        rr16 = rrp.tile([1, QC], f16, tag="rr16", name="rr16")
                        nc.scalar.copy(rr16[:], rr[:])
                        rbc = psp.tile([128, QC], f32, tag="big", name="big")
                        nc.tensor.matmul(
                            rbc[:], lhsT=ones_row[:], rhs=rr16[:],
                            start=True, stop=True)
                        an = anp.tile([64, QC], f16, tag="an", name="an")
                        nc.vector.tensor_mul(an[:], au[:], rbc[0:64, :])
                        res[h] = an
                    return res

                # Both pairs' pipelines interleaved; AV trails logits by 4
                # k-chunks so the PE never stalls on the exp/mask pipeline.
                # The previous q-block's output projection is emitted in the
                # middle of this block's pipeline so it fills PE gaps instead
                # of serializing at the block boundary.
                pend = {p: [] for p in range(PAIRS)}
                for kc in range(NKC):
                    ksl = slice(128 * kc, 128 * (kc + 1))
                    for pair in range(PAIRS):
                        lg2 = big()
                        for half in range(2):
                            psl = slice(64 * half, 64 * (half + 1))
                            nc.tensor.matmul(
                                lg2[:, half, :],
                                lhsT=khT[pair][psl, ksl],
                                rhs=qhT[pair][psl, qsl],
                                start=True, stop=True)
                        ex2 = exp_p.tile([128, 2, QC], f16, tag="ex", name="ex")
                        nc.scalar.activation(
                            ex2[:], lg2[:], Act.Exp, scale=0.125)
                        exm2 = exm_p.tile([128, 2, QC], f16, tag="exm",
                                          name="exm")
                        nc.vector.tensor_mul(
                            exm2[:], ex2[:],
                            mk[kc][:].rearrange("p (o q) -> p o q", o=1)
                            .to_broadcast((128, 2, QC)))
                        pend[pair].append((pair, kc, exm2))
                        if len(pend[pair]) > 4:
                            p_, dk, dexm = pend[pair].pop(0)
                            emit_av(p_, dk, dexm)
                    if kc == 6 and prev_wo is not None:
                        emit_wo(*prev_wo)
                        prev_wo = None
                attnN = {}
                for p in range(PAIRS):
                    for p_, dk, dexm in pend[p]:
                        emit_av(p_, dk, dexm)
                    attnN.update(normalize(p))
                prev_wo = (qcb, attnN)
            emit_wo(*prev_wo)

    nc.compile()
    return nc


def _get_program():
    if "nc" not in _CACHE:
        _CACHE["nc"] = _build()
    return _CACHE["nc"]


def _in_maps(q, k, v, mask, wq, bq, wk, bk, wv, bv, wo, bo):
    q = np.asarray(q, np.float32)
    k = np.asarray(k, np.float32)
    v = np.asarray(v, np.float32)
    mask = np.asarray(mask, np.float32)
    wq = np.asarray(wq, np.float32)
    wk = np.asarray(wk, np.float32)
    wv = np.asarray(wv, np.float32)
    wo = np.asarray(wo, np.float32)
    bq = np.asarray(bq, np.float32)
    bk = np.asarray(bk, np.float32)
    bv = np.asarray(bv, np.float32)
    assert np.all(bv == 0.0), "nonzero bv not supported by this kernel"

    maps = []
    xqT = [np.ascontiguousarray(q[b].T).astype(np.float16) for b in range(B)]
    xkT = [np.ascontiguousarray(k[b].T).astype(np.float16) for b in range(B)]
    xvT = [np.ascontiguousarray(v[b].T).astype(np.float16) for b in range(B)]
    m01 = [np.ascontiguousarray((1.0 - mask[b, 0]).T).astype(np.float16)
           for b in range(B)]
    for c in range(NCORES):
        b, g = divmod(c, GROUPS)
        cols = slice(FS * g, FS * (g + 1))
        maps.append({
            "xq": xqT[b], "xk": xkT[b], "xv": xvT[b],
            "wq": np.ascontiguousarray(wq[cols].T).astype(np.float16),
            "wk": np.ascontiguousarray(wk[cols].T).astype(np.float16),
            "wv": np.ascontiguousarray(wv[cols].T).astype(np.float16),
            "wo": np.ascontiguousarray(
                wo[:, cols].T.reshape(HG, DEPTH, DM)).astype(np.float16),
            "m01": m01[b],
            "bq": np.ascontiguousarray(bq[cols].reshape(2, 128).T),
            "bk": np.ascontiguousarray(bk[cols].reshape(2, 128).T),
        })
    return maps


def _run(maps, trace=False):
    from concourse.bass_utils import run_bass_kernel_spmd
    nc = _get_program()
    kwargs = {}
    if trace:
        kwargs = dict(trace=True, tmpdir=os.environ.get("KERNEL_TRACE_DIR"))
    return run_bass_kernel_spmd(nc, maps, list(range(NCORES)), **kwargs)


def kernel(q, k, v, mask, wq, bq, wk, bk, wv, bv, wo, bo):
    maps = _in_maps(q, k, v, mask, wq, bq, wk, bk, wv, bv, wo, bo)
    res = _run(maps)
    parts = [res.results[c]["part"] for c in range(NCORES)]
    bo = np.asarray(bo, np.float32)
    outb = [parts[GROUPS * b] + parts[GROUPS * b + 1]
            + parts[GROUPS * b + 2] + parts[GROUPS * b + 3] + bo
            for b in range(B)]
    return np.stack(outb, 0).astype(np.float32)


# revision 30
# speedup vs baseline: 1.0734x; 1.0453x over previous
# Multi-head attention (B=2, S=2048, D=1024, H=16) on 8 TRN2 NeuronCores.
#
# Sharding: core c -> batch b = c//4, head group g = c%4 (4 heads = 256
# features). Each core computes its heads' attention for its batch plus the
# row-parallel slice of the output projection; the host sums the 4 partials
# per batch (the all-reduce) and adds bo.
#
# Device math per core (layouts transposed so softmax needs no cross-
# partition reduce; all matmul operands fp16, accumulation fp32 in PSUM):
#   qhT[f, s] = wq_g @ q_b^T ; khT, vhT likewise     (fp16 matmuls)
#   vh[s, f]  = PE-transpose(vhT), slot layout per head: [1 | 0pad63 | v 64]
#   logitsT[k, q] = khT_h-slices^T @ qhT_h           (K=64, pairs row-packed
#                                                     into one 2-bank psum)
#   expT = exp(logitsT / 8) * (1 - mask)^T           (one wide ACT op, DVE
#                                                     mask mult, fp16)
#   av[:, q] = vh_slot^T @ expT                      (row 0 = denominator,
#                                                     rows 64:128 = head out)
#   attnN = av * broadcast(1/denom)                  (reciprocal + K=1 ones
#                                                     outer-product matmul)
#   partial[q, D] = attnN^T @ wo[:, g-cols]^T
import os
import numpy as np

B, S, DM, H, DEPTH = 2, 2048, 1024, 16, 64
NCORES = 8
GROUPS = 4            # head-groups per batch == cores per batch
HG = H // GROUPS      # heads per core
FS = HG * DEPTH       # features per core
QC = 512              # q-block (matmul free dim)
NQC = S // QC
NKC = S // 128        # k chunks
PAIRS = HG // 2
CCH = DM // 128       # contraction chunks for the projections

_CACHE = {}


def _build():
    import concourse.tile as tile
    from concourse import bacc, mybir

    dt = mybir.dt
    f32, f16 = dt.float32, dt.float16
    Act = mybir.ActivationFunctionType

    nc = bacc.Bacc("TRN2", target_bir_lowering=False, debug=False,
                   num_devices=NCORES)

    xq = nc.dram_tensor("xq", [DM, S], f16, kind="ExternalInput").ap()
    xk = nc.dram_tensor("xk", [DM, S], f16, kind="ExternalInput").ap()
    xv = nc.dram_tensor("xv", [DM, S], f16, kind="ExternalInput").ap()
    wqd = nc.dram_tensor("wq", [DM, FS], f16, kind="ExternalInput").ap()
    wkd = nc.dram_tensor("wk", [DM, FS], f16, kind="ExternalInput").ap()
    wvd = nc.dram_tensor("wv", [DM, FS], f16, kind="ExternalInput").ap()
    wod = nc.dram_tensor("wo", [HG, DEPTH, DM], f16, kind="ExternalInput").ap()
    m01 = nc.dram_tensor("m01", [S, S], f16, kind="ExternalInput").ap()
    bqd = nc.dram_tensor("bq", [128, 2], f32, kind="ExternalInput").ap()
    bkd = nc.dram_tensor("bk", [128, 2], f32, kind="ExternalInput").ap()
    out = nc.dram_tensor("part", [S, DM], f32, kind="ExternalOutput").ap()

    with tile.TileContext(nc) as tc:
        with (
            tc.tile_pool(name="xp", bufs=4) as xp,
            tc.tile_pool(name="wp", bufs=2) as wp,
            tc.tile_pool(name="wop", bufs=4) as wop,
            tc.tile_pool(name="qk", bufs=4) as qkp,
            tc.tile_pool(name="xv", bufs=8) as xvp,
            tc.tile_pool(name="vh", bufs=16) as vp,
            tc.tile_pool(name="mk", bufs=32) as mkp,
            tc.tile_pool(name="ex", bufs=6) as exp_p,
            tc.tile_pool(name="exm", bufs=10) as exm_p,
            tc.tile_pool(name="au", bufs=4) as aup,
            tc.tile_pool(name="an", bufs=8) as anp,
            tc.tile_pool(name="rr", bufs=4) as rrp,
            tc.tile_pool(name="os", bufs=4) as osp,
            tc.tile_pool(name="cst", bufs=4) as cst,
            tc.tile_pool(name="ps", bufs=4, space="PSUM") as psp,
        ):
            def big():
                return psp.tile([128, 2, QC], f32, tag="big", name="big")

            # weights, in use-order (wq first so the PE starts ASAP)
            wq_t = wp.tile([128, CCH, FS], f16, tag="w", name="w")
            nc.sync.dma_start(wq_t[:], wqd.rearrange("(c p) f -> p c f", p=128))
            bq_t = cst.tile([128, 2], f32, tag="bias", name="bias")
            nc.sync.dma_start(bq_t[:], bqd[:])
            wk_t = wp.tile([128, CCH, FS], f16, tag="w", name="w")
            nc.sync.dma_start(wk_t[:], wkd.rearrange("(c p) f -> p c f", p=128))
            bk_t = cst.tile([128, 2], f32, tag="bias", name="bias")
            nc.sync.dma_start(bk_t[:], bkd[:])
            wv_t = wp.tile([128, CCH, FS], f16, tag="w", name="w")
            nc.sync.dma_start(wv_t[:], wvd.rearrange("(c p) f -> p c f", p=128))
            ones_row = cst.tile([1, 128], f16, tag="ones", name="ones")
            nc.vector.memset(ones_row[:], 1.0)
            wo_t = []
            for h in range(HG):
                t = wop.tile([DEPTH, DM], f16, tag="wo", name="wo")
                nc.sync.dma_start(t[:], wod[h])
                wo_t.append(t)

            # ---- projections (q/k produce transposed [FS, S] fp16) ----
            qhT = [qkp.tile([128, S], f16, tag="qk", name="qk") for _ in range(2)]
            khT = [qkp.tile([128, S], f16, tag="qk", name="qk") for _ in range(2)]

            def proj(xd, w_t, dst, bias):
                pst = [big() for _ in range(4)]
                for c in range(CCH):
                    xt = xp.tile([128, S], f16, tag="x", name="x")
                    nc.sync.dma_start(xt[:], xd[128 * c:128 * (c + 1), :])
                    for m in range(2):
                        for qs in range(4):
                            i = m * 4 + qs
                            nc.tensor.matmul(
                                pst[i // 2][:, i % 2, :],
                                lhsT=w_t[:, c, 128 * m:128 * (m + 1)],
                                rhs=xt[:, QC * qs:QC * (qs + 1)],
                                start=(c == 0), stop=(c == CCH - 1),
                            )
                # copybacks on ScalarE: it is idle during the projection
                # phase while VectorE is the attention-phase co-bottleneck
                for m in range(2):
                    for qs in range(4):
                        i = m * 4 + qs
                        nc.scalar.add(
                            dst[m][:, QC * qs:QC * (qs + 1)],
                            pst[i // 2][:, i % 2, :], bias[:, m:m + 1])

            proj(xq, wq_t, qhT, bq_t)
            proj(xk, wk_t, khT, bk_t)

            # v projection straight into vh [k, feat] layout: kr-outer with
            # xv resident, one transient psum bank at a time (so it overlaps
            # with the start of attention).  vh slot layout (128 wide): col 0
            # = ones (denominator), cols 1:64 zero pad, cols 64:128 = v data
            # -> av row 0 = denominator, rows 64:128 = head output.
            xv_t = []
            for c in range(CCH):
                t = xvp.tile([128, S], f16, tag="xv", name="xv")
                nc.sync.dma_start(t[:], xv[128 * c:128 * (c + 1), :])
                xv_t.append(t)
            vh = [vp.tile([128, HG, 128], f16, tag="vh", name="vh")
                  for _ in range(NKC)]
            for kr in range(NKC):
                pv = psp.tile([128, 256], f32, tag="big", name="big")
                for c in range(CCH):
                    nc.tensor.matmul(
                        pv[:], lhsT=xv_t[c][:, 128 * kr:128 * (kr + 1)],
                        rhs=wv_t[:, c, :],
                        start=(c == 0), stop=(c == CCH - 1))
                nc.vector.memset(vh[kr][:, :, 0:1], 1.0)
                nc.vector.memset(vh[kr][:, :, 1:64], 0.0)
                nc.scalar.copy(
                    vh[kr][:, :, 64:128],
                    pv.rearrange("p (h d) -> p h d", d=DEPTH))

            # ---- attention + output projection, per q-block ----
            def emit_wo(qcb, attnN):
                for qm in range(QC // 128):
                    row = slice(128 * (4 * qcb + qm), 128 * (4 * qcb + qm + 1))
                    po = big()
                    for dn in range(2):
                        dsl = slice(512 * dn, 512 * (dn + 1))
                        for h in range(HG):
                            nc.tensor.matmul(
                                po[:, dn, :],
                                lhsT=attnN[h][:, 128 * qm:128 * (qm + 1)],
                                rhs=wo_t[h][:, dsl],
                                start=(h == 0), stop=(h == HG - 1))
                    ot = osp.tile([128, 2, 512], f32, tag="os", name="os")
                    nc.scalar.copy(ot[:, 0, :], po[:, 0, :])
                    nc.vector.tensor_copy(ot[:, 1, :], po[:, 1, :])
                    nc.sync.dma_start(
                        out[row, :].rearrange("p (o q) -> p o q", o=2), ot[:])

            prev_wo = None
            prev_norm = None
            for qcb in range(NQC):
                qsl = slice(QC * qcb, QC * (qcb + 1))
                mk = []
                for kc in range(NKC):
                    t = mkp.tile([128, QC], f16, tag="mk", name="mk")
                    nc.sync.dma_start(
                        t[:], m01[128 * kc:128 * (kc + 1), qsl])
                    mk.append(t)

                av2 = [big() for _ in range(PAIRS)]   # halves = heads A/B

                def emit_av(pair, dk, exm2, av2=av2):
                    for half in range(2):
                        nc.tensor.matmul(
                            av2[pair][:, half, :],
                            lhsT=vh[dk][:, 2 * pair + half, :],
                            rhs=exm2[:, half, :],
                            start=(dk == 0), stop=(dk == NKC - 1),
                            skip_group_check=True)

                def normalize_start(pair, av2=av2):
                    # only the two ops that must read av2 run at the q-block
                    # boundary (frees the accumulator PSUM banks quickly);
                    # the rest is deferred into the next block's pipeline
                    out = []
                    for half in range(2):
                        h = 2 * pair + half
                        au = aup.tile([64, QC], f32, tag="au", name="au")
                        nc.scalar.copy(au[:], av2[pair][64:128, half, :])
                        rr = rrp.tile([1, QC], f32, tag="rr", name="rr")
                        nc.vector.reciprocal_approx_fast(
                            rr[:], av2[pair][0:1, half, :])
                        out.append((h, au, rr))
                    return out

                # Both pairs' pipelines interleaved; AV trails logits by 4
                # k-chunks so the PE never stalls on the exp/mask pipeline.
                # The previous q-block's output projection is emitted in the
                # middle of this block's pipeline so it fills PE gaps instead
                # of serializing at the block boundary.
                pend = {p: [] for p in range(PAIRS)}
                for kc in range(NKC):
                    ksl = slice(128 * kc, 128 * (kc + 1))
                    for pair in range(PAIRS):
                        lg2 = big()
                        for half in range(2):
                            psl = slice(64 * half, 64 * (half + 1))
                            nc.tensor.matmul(
                                lg2[:, half, :],
                                lhsT=khT[pair][psl, ksl],
                                rhs=qhT[pair][psl, qsl],
                                start=True, stop=True)
                        ex2 = exp_p.tile([128, 2, QC], f16, tag="ex", name="ex")
                        nc.scalar.activation(
                            ex2[:], lg2[:], Act.Exp, scale=0.125)
                        exm2 = exm_p.tile([128, 2, QC], f16, tag="exm",
                                          name="exm")
                        nc.vector.tensor_mul(
                            exm2[:], ex2[:],
                            mk[kc][:].rearrange("p (o q) -> p o q", o=1)
                            .to_broadcast((128, 2, QC)))
                        pend[pair].append((pair, kc, exm2))
                        if len(pend[pair]) > 4:
                            p_, dk, dexm = pend[pair].pop(0)
                            emit_av(p_, dk, dexm)
                    if kc == 2 and prev_norm is not None:
                        pq, items = prev_norm
                        attnN = {}
                        for h, au, rr in items:
                            rr16 = rrp.tile([1, QC], f16, tag="rr16",
                                            name="rr16")
                            nc.scalar.copy(rr16[:], rr[:])
                            rbc = psp.tile([128, QC], f32, tag="big",
                                           name="big")
                            nc.tensor.matmul(
                                rbc[:], lhsT=ones_row[:], rhs=rr16[:],
                                start=True, stop=True)
                            an = anp.tile([64, QC], f16, tag="an", name="an")
                            nc.vector.tensor_mul(an[:], au[:], rbc[0:64, :])
                            attnN[h] = an
                        prev_wo = (pq, attnN)
                        prev_norm = None
                    if kc == 6 and prev_wo is not None:
                        emit_wo(*prev_wo)
                        prev_wo = None
                items = []
                for p in range(PAIRS):
                    for p_, dk, dexm in pend[p]:
                        emit_av(p_, dk, dexm)
                    items.extend(normalize_start(p))
                prev_norm = (qcb, items)
            pq, items = prev_norm
            attnN = {}
            for h, au, rr in items:
                rr16 = rrp.tile([1, QC], f16, tag="rr16", name="rr16")
                nc.scalar.copy(rr16[:], rr[:])
                rbc = psp.tile([128, QC], f32, tag="big", name="big")
                nc.tensor.matmul(rbc[:], lhsT=ones_row[:], rhs=rr16[:],
                                 start=True, stop=True)
                an = anp.tile([64, QC], f16, tag="an", name="an")
                nc.vector.tensor_mul(an[:], au[:], rbc[0:64, :])
                attnN[h] = an
            emit_wo(pq, attnN)

    nc.compile()
    return nc


def _get_program():
    if "nc" not in _CACHE:
        _CACHE["nc"] = _build()
    return _CACHE["nc"]


def _in_maps(q, k, v, mask, wq, bq, wk, bk, wv, bv, wo, bo):
    q = np.asarray(q, np.float32)
    k = np.asarray(k, np.float32)
    v = np.asarray(v, np.float32)
    mask = np.asarray(mask, np.float32)
    wq = np.asarray(wq, np.float32)
    wk = np.asarray(wk, np.float32)
    wv = np.asarray(wv, np.float32)
    wo = np.asarray(wo, np.float32)
    bq = np.asarray(bq, np.float32)
    bk = np.asarray(bk, np.float32)
    bv = np.asarray(bv, np.float32)
    assert np.all(bv == 0.0), "nonzero bv not supported by this kernel"

    maps = []
    xqT = [np.ascontiguousarray(q[b].T).astype(np.float16) for b in range(B)]
    xkT = [np.ascontiguousarray(k[b].T).astype(np.float16) for b in range(B)]
    xvT = [np.ascontiguousarray(v[b].T).astype(np.float16) for b in range(B)]
    m01 = [np.ascontiguousarray((1.0 - mask[b, 0]).T).astype(np.float16)
           for b in range(B)]
    for c in range(NCORES):
        b, g = divmod(c, GROUPS)
        cols = slice(FS * g, FS * (g + 1))
        maps.append({
            "xq": xqT[b], "xk": xkT[b], "xv": xvT[b],
            "wq": np.ascontiguousarray(wq[cols].T).astype(np.float16),
            "wk": np.ascontiguousarray(wk[cols].T).astype(np.float16),
            "wv": np.ascontiguousarray(wv[cols].T).astype(np.float16),
            "wo": np.ascontiguousarray(
                wo[:, cols].T.reshape(HG, DEPTH, DM)).astype(np.float16),
            "m01": m01[b],
            "bq": np.ascontiguousarray(bq[cols].reshape(2, 128).T),
            "bk": np.ascontiguousarray(bk[cols].reshape(2, 128).T),
        })
    return maps


def _run(maps, trace=False):
    from concourse.bass_utils import run_bass_kernel_spmd
    nc = _get_program()
    kwargs = {}
    if trace:
        kwargs = dict(trace=True, tmpdir=os.environ.get("KERNEL_TRACE_DIR"))
    return run_bass_kernel_spmd(nc, maps, list(range(NCORES)), **kwargs)


def kernel(q, k, v, mask, wq, bq, wk, bk, wv, bv, wo, bo):
    maps = _in_maps(q, k, v, mask, wq, bq, wk, bk, wv, bv, wo, bo)
    res = _run(maps)
    parts = [res.results[c]["part"] for c in range(NCORES)]
    bo = np.asarray(bo, np.float32)
    outb = [parts[GROUPS * b] + parts[GROUPS * b + 1]
            + parts[GROUPS * b + 2] + parts[GROUPS * b + 3] + bo
            for b in range(B)]
    return np.stack(outb, 0).astype(np.float32)
